# revision 26
# baseline (speedup 1.0000x reference)
"""Trainium2 Bass kernel for DecomposedShiftNet (V3: fp8 DoubleRow h2/logits via
cast-DMA, feature-major backend, batched 128-free tail).

Computation (per batch row b, bits=64, H=512):
  shift_soft = softmax(MLP_sd(shift_bits))                       # [64]
  h1[i,:]  = relu(ix_w1[i] + shift_soft @ ix_w1[64:] + ix_b1)    # [64, 512]
  h2       = relu(h1 @ ix_w2 + ix_b2)                            # [64, 512]
  p[i,k]   = softmax(h2 @ ix_w3 + ix_b3)[i, :64]                 # [64, 64]
  pointed  = p @ a_bits[b]                                       # [64]
  vh[i,:]  = relu(v_w1[i] + shift_soft @ v_w1[64:] + v_b1)       # [64, 256]
  valid    = vh @ v_w2 + v_b2                                    # [64]
  out[b]   = pointed * sigmoid(valid)

Strategy: pure data parallel over 8 cores (256 batch rows each). On-core
layout is feature-major: activations [features(part), cols(free)], cols =
2 positions x 256 batch = 512 per block, 32 blocks, software pipelined.

Measured-cost-driven engine plan (per block, targets ~2.9us/engine):
  - DVE: h1 build bf16 (8x tensor_scalar, 2-byte 2x mode ~205ns), vh build
    (4x), exp upper-half multiply by a_bits (bf16), tail bits.
  - Act: the only engine with fast fp8 output -> all 4 h2 relu+bias evicts
    (PSUM f32 -> fp8), the exp evict (-> bf16), tail tanh.
  - Pool (GpSimd): useless for compute (~3.9us/op measured) but issues the
    casting DMAs: h1 bf16 -> fp8 conversion runs on the DMA engines
    (descriptor-generated by Pool, ~620ns issue), plus stash DMAs.
  - PE: h2 as fp8 DoubleRow matmuls (K=256/pass, measured 216ns @ N=512 =
    2x bf16 FLOPs), logits as 2 DoubleRow matmuls with ix_w3 duplicated to
    both array halves, den/num/valid reductions as small matmuls.
  - Scales: h1 x64, ix_w2 x16 -> h2 PSUM arrives at scale 1024, so the relu
    evict needs no rescale; ix_w3 x16 -> exp evict scale 1/16384.
  - Softmax denominators stash to SBUF via DMA in a [4*blk, 128] layout so
    the tail runs on 128-free tiles: reciprocal_approx_fast + 2 DVE ops +
    tanh, then PE transposes emit batch-major output.
"""

import sys

import ml_dtypes
import numpy as np

for _p in ("/opt/trn_rl_repo",):
    if _p not in sys.path:
        sys.path.insert(0, _p)

import concourse.bacc as bacc
import concourse.bass as bass
import concourse.tile as tile
from concourse import bass_utils, mybir

F32 = mybir.dt.float32
F32R = mybir.dt.float32r
BF16 = mybir.dt.bfloat16
F8 = mybir.dt.float8e4
AF = mybir.ActivationFunctionType
OP = mybir.AluOpType
DR = mybir.MatmulPerfMode.DoubleRow

B, BITS, H = 2048, 64, 512
NCORES = 8
BC = B // NCORES  # 256 rows per core
NBLK = BITS // 2  # 32 blocks of 2 positions
NB = 2 * BC  # 512 free columns per block
HV = H // 2  # validity hidden = 256

S1 = 64.0  # h1 fp8 scale
SW2 = 16.0  # ix_w2 fp8 scale
S2 = S1 * SW2  # h2 fp8 scale = 1024
SW3 = 16.0  # ix_w3 fp8 scale
EXPS = 1.0 / (S2 * SW3)  # scale applied inside the exp evict

# tail segments (blocks): sized so early segments overlap the main loop
SEGS = [(0, 16), (16, 8), (24, 4), (28, 4)]


def to_f32r_np(a):
    u = np.ascontiguousarray(a, dtype=np.float32).view(np.uint32)
    r = (u + 0x7FF + ((u >> 12) & 1)) & np.uint32(0xFFFFF000)
    return r.view(np.float32)


# name -> (shape, dtype code)
_INPUTS = {
    "sbT": ((BITS, BC), "bf16"),
    "wsd1": ((BITS, H), "bf16"),
    "sdb1": ((128, 4), "f32"),
    "wsd2": ((128, 4, H), "bf16"),
    "sdb2": ((128, 4), "f32"),
    "wsd3": ((128, 4, BITS), "bf16"),
    "sdb3": ((BITS, 1), "f32"),
    "ones64": ((BITS, 1), "bf16"),
    "ones1": ((1, BITS), "f32"),
    "wixb": ((BITS, H), "f32r"),
    "pb": ((128, 4, BITS), "f32"),
    "wvb": ((BITS, HV), "f32r"),
    "vpb": ((128, 2, BITS), "f32"),
    "wix2": ((128, 4, H), "f8"),
    "ixb2": ((128, 4), "f32"),
    "wix3d": ((128, 4, 128), "f8"),
    "ixb3e": ((128, 1), "f32"),
    "abT": ((128, BC), "bf16"),
    "dn_w3": ((128, 32), "bf16"),
    "wv2a": ((128, 32), "bf16"),
    "wv2b": ((128, 32), "bf16"),
    "vb2h": ((64, 1), "f32"),
    "ident": ((64, 64), "f32"),
}

DT = {"f32": F32, "f32r": F32R, "bf16": BF16, "f8": F8}


def _emit(nc, tc, I, out):
    import contextlib

    ctx = contextlib.ExitStack()
    with ctx:
        const = ctx.enter_context(tc.tile_pool(name="const", bufs=1))
        work = ctx.enter_context(tc.tile_pool(name="work", bufs=9))
        psA = ctx.enter_context(tc.tile_pool(name="psA", bufs=2, space="PSUM"))
        psB = ctx.enter_context(tc.tile_pool(name="psB", bufs=1, space="PSUM"))
        psD = ctx.enter_context(tc.tile_pool(name="psD", bufs=2, space="PSUM"))

        # ---------------- load everything (critical-path order) ---------------
        T = {}

        def load(name, eng):
            shape, code = _INPUTS[name]
            t = const.tile(list(shape), DT[code], tag=name, name=name)
            eng.dma_start(out=t, in_=I[name])
            T[name] = t
            return t

        # critical-path (shift decoder) loads on the HW queue; the main-loop
        # weights go through gpsimd's software DGE, which is idle here.
        for name in ("sbT", "wsd1", "sdb1", "wsd2", "sdb2", "wsd3", "sdb3",
                     "ones64", "ones1", "wixb", "wvb", "pb", "vpb"):
            load(name, nc.sync)
        for name in ("wix2", "ixb2", "wix3d", "ixb3e", "abT", "dn_w3",
                     "wv2a", "wv2b", "vb2h", "ident"):
            load(name, nc.gpsimd)

        sbT, ones64, ones1 = T["sbT"], T["ones64"], T["ones1"]
        wsd1, wsd2, wsd3 = T["wsd1"], T["wsd2"], T["wsd3"]
        sdb1, sdb2, sdb3 = T["sdb1"], T["sdb2"], T["sdb3"]
        wixb, pb, wvb, vpb = T["wixb"], T["pb"], T["wvb"], T["vpb"]
        wix2, ixb2, wix3d, ixb3e = T["wix2"], T["ixb2"], T["wix3d"], T["ixb3e"]
        abT, dn_w3 = T["abT"], T["dn_w3"]
        wv2a, wv2b = T["wv2a"], T["wv2b"]
        vb2h, ident = T["vb2h"], T["ident"]

        # ---------------- shift decoder MLP (feature-major, N=256) ------------
        hsd1 = const.tile([128, 4, BC], BF16, tag="hsd1", name="hsd1")
        for m in range(4):
            ps = psA.tile([128, 2, NB], F32, tag="mm", name="mmps")
            nc.tensor.matmul(ps[:, 0, :BC], wsd1[:, m * 128:(m + 1) * 128], sbT,
                             start=True, stop=True)
            nc.scalar.activation(hsd1[:, m, :], ps[:, 0, :BC], AF.Relu,
                                 bias=sdb1[:, m:m + 1])
        hsd2 = const.tile([128, 4, BC], BF16, tag="hsd2", name="hsd2")
        for m in range(4):
            ps = psA.tile([128, 2, NB], F32, tag="mm", name="mmps")
            for k in range(4):
                nc.tensor.matmul(ps[:, 0, :BC], wsd2[:, k, m * 128:(m + 1) * 128],
                                 hsd1[:, k, :], start=(k == 0), stop=(k == 3))
            nc.scalar.activation(hsd2[:, m, :], ps[:, 0, :BC], AF.Relu,
                                 bias=sdb2[:, m:m + 1])
        ps3 = psA.tile([128, 2, NB], F32, tag="mm", name="mmps")
        for k in range(4):
            nc.tensor.matmul(ps3[:BITS, 0, :BC], wsd3[:, k, :], hsd2[:, k, :],
                             start=(k == 0), stop=(k == 3))
        exp_sd = const.tile([BITS, BC], BF16, tag="exp_sd", name="exp_sd")
        nc.scalar.activation(exp_sd, ps3[:BITS, 0, :BC], AF.Exp, bias=sdb3)

        # softmax normalize: denom across partitions via ones-matmul, then
        # reciprocal + K=1 outer-product broadcast back over partitions.
        psd = psD.tile([128, NB], F32, tag="dn", name="dnps")
        nc.tensor.matmul(psd[0:1, :BC], ones64, exp_sd, start=True, stop=True)
        rec_sd = const.tile([1, BC], F32, tag="rec_sd", name="rec_sd")
        nc.vector.reciprocal_approx_fast(out=rec_sd, in_=psd[0:1, :BC])
        psb = psD.tile([128, NB], F32, tag="dn", name="dnps")
        nc.tensor.matmul(psb[:BITS, :BC], ones1, rec_sd, start=True, stop=True)
        shift_soft = const.tile([BITS, BC], F32R, tag="ssoft", name="ssoft")
        with nc.allow_low_precision(reason="softmax weights in f32r (12-bit mantissa) is plenty"):
            nc.vector.tensor_tensor(shift_soft, exp_sd, psb[:BITS, :BC], OP.mult)

        # sp = 64 * shift_part  [H, BC] bf16 ; vs = v_shift [HV, BC] bf16
        sp = const.tile([128, 4, BC], BF16, tag="sp", name="sp")
        for m in range(4):
            ps = psA.tile([128, 2, NB], F32, tag="mm", name="mmps")
            nc.tensor.matmul(ps[:, 0, :BC], wixb[:, m * 128:(m + 1) * 128],
                             shift_soft, start=True, stop=True)
            nc.vector.tensor_copy(out=sp[:, m, :], in_=ps[:, 0, :BC])
        vs = const.tile([128, 2, BC], BF16, tag="vs", name="vs")
        for m in range(2):
            ps = psA.tile([128, 2, NB], F32, tag="mm", name="mmps")
            nc.tensor.matmul(ps[:, 0, :BC], wvb[:, m * 128:(m + 1) * 128],
                             shift_soft, start=True, stop=True)
            nc.vector.tensor_copy(out=vs[:, m, :], in_=ps[:, 0, :BC])

        # stash tiles per segment: [4*nblk, 128] layout (row = 4j + q with
        # q = 2*h + bhalf, cols = 128 batch within half) so every tail op has
        # free size 128 instead of 512.
        stash = []
        for si, (s0, sn) in enumerate(SEGS):
            stash.append({
                "d": const.tile([4 * sn, 128], F32, tag=f"std{si}", name=f"std{si}"),
                "n": const.tile([4 * sn, 128], F32, tag=f"stn{si}", name=f"stn{si}"),
                "v": const.tile([4 * sn, 128], F32, tag=f"stv{si}", name=f"stv{si}"),
            })
        seg_of = {}
        for si, (s0, sn) in enumerate(SEGS):
            for j in range(s0, s0 + sn):
                seg_of[j] = (si, j - s0)

        obm = [const.tile([128, BITS], F32, tag=f"obm{h}", name=f"obm{h}")
               for h in range(2)]

        # ---------------- main loop (software pipelined) ----------------
        st = {}
        grp = {}

        def stage_front_a(j):
            """first h1 ops (DVE queue warm-up: always ready)."""
            d = st[j] = {}
            h1b = d["h1b"] = work.tile([128, 4, NB], BF16, tag="h1b", name="h1b")
            for c, h in ((0, 0), (0, 1), (1, 0)):
                i = 2 * j + h
                nc.vector.tensor_scalar(
                    h1b[:, c, h * BC:(h + 1) * BC], sp[:, c, :],
                    pb[:, c, i:i + 1], 0.0, OP.add, OP.max)

        def stage_front_b(j):
            """rest of h1 + cast-DMA to fp8 (Pool->DMA HW) + vh (DVE)."""
            d = st[j]
            h1b = d.pop("h1b")
            for c, h in ((1, 1), (2, 0), (2, 1), (3, 0), (3, 1)):
                i = 2 * j + h
                nc.vector.tensor_scalar(
                    h1b[:, c, h * BC:(h + 1) * BC], sp[:, c, :],
                    pb[:, c, i:i + 1], 0.0, OP.add, OP.max)
            h1 = d["h1"] = work.tile([128, 4, NB], F8, tag="h1", name="h1")
            nc.gpsimd.dma_start(out=h1, in_=h1b)  # casting DMA bf16 -> fp8
            vh = d["vh"] = work.tile([128, 2, NB], BF16, tag="vh", name="vh")
            for c in range(2):
                for h in range(2):
                    i = 2 * j + h
                    nc.vector.tensor_scalar(
                        vh[:, c, h * BC:(h + 1) * BC], vs[:, c, :],
                        vpb[:, c, i:i + 1], 0.0, OP.add, OP.max)

        def stage_mms(jA, half):
            """h2 m-chunks (2*half, 2*half+1) for blocks jA, jA+1: DoubleRow
            fp8 matmuls into paired PSUM banks; the same m-chunk of both
            blocks shares its bias, so ONE Act relu evict [128, 2*512]
            handles the pair."""
            dA, dB = st[jA], st[jA + 1]
            if half == 0:
                h2p = work.tile([128, 2, 4, NB], F8, tag="h2", name="h2")
                dA["h2p"] = h2p
                dA["h2"] = h2p[:, 0]
                dB["h2"] = h2p[:, 1]
            h2p = dA["h2p"]
            for m in (2 * half, 2 * half + 1):
                ps = psA.tile([128, 2, NB], F32, tag="mm", name="mmps")
                for blk, d in ((0, dA), (1, dB)):
                    h1 = d["h1"]
                    for k2 in range(2):
                        nc.tensor.matmul(
                            ps[:, blk, :],
                            wix2[:, 2 * k2:2 * k2 + 2, m * 128:(m + 1) * 128],
                            h1[:, 2 * k2:2 * k2 + 2, :],
                            start=(k2 == 0), stop=(k2 == 1), perf_mode=DR)
                nc.scalar.activation(
                    h2p.rearrange("p a c f -> p c a f")[:, m, :, :], ps,
                    AF.Relu, bias=ixb2[:, m:m + 1])

        def stage_logits(jA, k2):
            """dup logits k-tile pair k2 for blocks jA, jA+1 into the shared
            pair PSUM tile."""
            dA, dB = st[jA], st[jA + 1]
            if k2 == 0:
                dA["plp"] = psB.tile([128, 2, NB], F32, tag="lg", name="lgps")
            plp = dA["plp"]
            for blk, d in ((0, dA), (1, dB)):
                nc.tensor.matmul(plp[:, blk, :], wix3d[:, 2 * k2:2 * k2 + 2, :],
                                 d["h2"][:, 2 * k2:2 * k2 + 2, :],
                                 start=(k2 == 0), stop=(k2 == 1), perf_mode=DR)

        def stage_exp_pair(jA):
            """paired exp evict for blocks jA, jA+1 (inputs 2 iters old)."""
            dA, dB = st[jA], st[jA + 1]
            ep = work.tile([128, 2, NB], BF16, tag="exp", name="exp2")
            dA["exp"] = ep[:, 0]
            dB["exp"] = ep[:, 1]
            nc.scalar.activation(ep.rearrange("p a f -> p (a f)"),
                                 dA.pop("plp").rearrange("p a f -> p (a f)"),
                                 AF.Exp, bias=ixb3e, scale=EXPS)

        def stage_expmult(j):
            """upper half of exp2 *= a_bits.T (one block)."""
            exp2 = st[j]["exp"]
            up = exp2[64:128, :].rearrange("p (h b) -> p h b", h=2)
            nc.vector.tensor_tensor(
                up, up, abT[64:128, :].unsqueeze(1).broadcast_to([64, 2, BC]),
                OP.mult)

        def stage_red(j):
            d = st[j]
            exp2, vh = d["exp"], d["vh"]
            # den/num/valid rows: 3-row region per block, 4 blocks per PSUM
            # bank at base partitions 0/32/64/96, accumulated via zero-padded
            # lhsT columns; one Act evict per group of 4 blocks.
            g, rr = j // 4, j % 4
            if rr == 0:
                grp[g] = psD.tile([128, NB], F32, tag="dn", name="dnps")
            psd = grp[g]
            base = 32 * rr
            nc.tensor.matmul(psd[base:base + 32, :], dn_w3, exp2,
                             start=True, stop=False, tile_position=(0, base))
            nc.tensor.matmul(psd[base:base + 32, :], wv2a, vh[:, 0, :],
                             start=False, stop=False, tile_position=(0, base))
            nc.tensor.matmul(psd[base:base + 32, :], wv2b, vh[:, 1, :],
                             start=False, stop=True, tile_position=(0, base))
            del st[j]

        def stage_stash(j):
            g, rr = j // 4, j % 4
            if rr == 3:
                psd = grp.pop(g)
                ge = work.tile([128, NB], F32, tag="ge", name="ge")
                nc.vector.tensor_copy(out=ge[0:99, :], in_=psd[0:99, :])
                si, j0 = seg_of[4 * g]
                sg = stash[si]
                for i in range(4):
                    jj = j0 + i
                    for t, key in ((0, "d"), (1, "n"), (2, "v")):
                        row = ge[32 * i + t:32 * i + t + 1, :]
                        src = row.rearrange("p (q b) -> p q b", q=4)
                        dst = sg[key][4 * jj:4 * jj + 4, :]
                        if t == 0 or g >= NBLK // 4 - 3:
                            # last groups avoid Pool so its slow software
                            # DGE drain overlaps earlier work
                            nc.sync.dma_start(out=dst, in_=src)
                        else:
                            nc.gpsimd.dma_start(out=dst, in_=src)

        tails = {}

        def tail_compute(si):
            """outv = (tanh(0.5 v + 0.5 b2) + 1) * (0.5 * n / d), all [*,128]."""
            sn = SEGS[si][1]
            sg = stash[si]
            rd = const.tile([4 * sn, 128], F32, tag=f"trd{si}", name=f"trd{si}")
            nc.vector.reciprocal_approx_fast(out=rd, in_=sg["d"])
            nr = const.tile([4 * sn, 128], F32, tag=f"tnr{si}", name=f"tnr{si}")
            nc.vector.tensor_tensor(nr, sg["n"], rd, OP.mult)
            th = const.tile([4 * sn, 128], F32, tag=f"tth{si}", name=f"tth{si}")
            nc.scalar.activation(th, sg["v"], AF.Tanh, bias=vb2h[0:4 * sn, :],
                                 scale=0.5)
            outv = tails[si] = const.tile([4 * sn, 128], F32, tag=f"tov{si}",
                                          name=f"tov{si}")
            nc.vector.scalar_tensor_tensor(outv, th, 1.0, nr, OP.add, OP.mult)

        def tail_transpose(si):
            s0, sn = SEGS[si]
            outv = tails.pop(si)
            # PE transpose in chunks of up to 64 partitions (16 blocks each)
            csz = min(64, 4 * sn)
            nbl = csz // 4  # blocks per chunk
            for ch in range((4 * sn) // csz):
                ps = psD.tile([128, NB], F32, tag="dn", name="dnps")
                nc.tensor.transpose(ps[:, 0:csz],
                                    outv[ch * csz:(ch + 1) * csz, :],
                                    ident[0:csz, 0:csz])
                # ps[bb, r] with r = 4j' + 2h + bh (j' within chunk)
                pv = ps[:, 0:csz].rearrange("p (jj hh bh) -> p jj hh bh",
                                            jj=nbl, hh=2)
                base = 2 * (s0 + ch * nbl)
                for bh in range(2):
                    ov = obm[bh][:, base:base + 2 * nbl].rearrange(
                        "p (jj hh) -> p jj hh", jj=nbl)
                    nc.vector.tensor_copy(out=ov, in_=pv[:, :, :, bh])

        def emit_out(si):
            s0, sn = SEGS[si]
            for half in range(2):
                nc.sync.dma_start(
                    out=out[half * 128:(half + 1) * 128, 2 * s0:2 * (s0 + sn)],
                    in_=obm[half][:, 2 * s0:2 * (s0 + sn)])

        # stagger: every producer->consumer crosses >=1 iteration so no
        # engine queue head ever waits on same-iteration work.
        #   front(b)@b | mms m01(p)@p+2, m23@p+3 | logits k2=0(p)@p+4, k2=1@p+5
        #   exp-pair(p)@p+6 | expmult(b)@b+6(even b)/b+5.. uniform @j-6
        #   red(b)@b+7 | stash@b+8 | group g stash lands @4g+11
        for j in range(NBLK + 12):
            if j < NBLK:
                stage_front_a(j)          # DVE warm-up (always ready)
            if j % 2 == 0 and 6 <= j <= NBLK + 4:
                stage_exp_pair(j - 6)     # Act queue head (inputs 1-2 iters old)
            if 6 <= j < NBLK + 6:
                stage_expmult(j - 6)      # DVE (its exp ran this or last iter)
            if j % 2 == 0 and 2 <= j <= NBLK:
                stage_mms(j - 2, 0)       # PE bulk + Act paired relu evicts
            if j % 2 == 1 and 3 <= j <= NBLK + 1:
                stage_mms(j - 3, 1)
            if j % 2 == 0 and 4 <= j <= NBLK + 2:
                stage_logits(j - 4, 0)
            if j % 2 == 1 and 5 <= j <= NBLK + 3:
                stage_logits(j - 5, 1)
            if 7 <= j < NBLK + 7:
                stage_red(j - 7)          # PE tail (inputs >=1 iter old)
            if j < NBLK:
                stage_front_b(j)          # DVE bulk + Pool cast
            if 8 <= j < NBLK + 8:
                stage_stash(j - 8)        # DVE queue tail + DMA queues
            if j == 24:
                tail_compute(0)
            if j == 26:
                tail_transpose(0)
                emit_out(0)
            if j == 32:
                tail_compute(1)
            if j == 34:
                tail_transpose(1)
                emit_out(1)
            if j == 36:
                tail_compute(2)
            if j == 38:
                tail_transpose(2)
                emit_out(2)
        tail_compute(3)
        tail_transpose(3)
        emit_out(3)


def build_program():
    nc = bacc.Bacc("TRN2", target_bir_lowering=False, debug=False, enable_asserts=False)
    I = {}
    for name, (shape, code) in _INPUTS.items():
        I[name] = nc.dram_tensor(name, list(shape), DT[code], kind="ExternalInput").ap()
    out = nc.dram_tensor("out", [BC, BITS], F32, kind="ExternalOutput").ap()

    with tile.TileContext(nc) as tc:
        _emit(nc, tc, I, out)
    nc.compile()
    return nc


_NC = None


def _get_program():
    global _NC
    if _NC is None:
        _NC = build_program()
    return _NC


def make_in_maps(inputs):
    """Shard batch tensors across cores; replicate weights. All layout prep
    (transposes, tiling, bias folding, fp8/bf16 pre-scaling) happens here."""
    f = {k: np.ascontiguousarray(np.asarray(v, dtype=np.float32)) for k, v in inputs.items()}
    r = to_f32r_np
    bf = ml_dtypes.bfloat16
    f8 = ml_dtypes.float8_e4m3

    shared = {
        "wsd1": f["sd_w1"].astype(bf),
        "sdb1": f["sd_b1"].reshape(4, 128).T.astype(np.float32),
        "wsd2": f["sd_w2"].reshape(4, 128, H).transpose(1, 0, 2).astype(bf),
        "sdb2": f["sd_b2"].reshape(4, 128).T.astype(np.float32),
        "wsd3": f["sd_w3"].reshape(4, 128, BITS).transpose(1, 0, 2).astype(bf),
        "sdb3": f["sd_b3"][:, None].astype(np.float32),
        "ones64": np.ones((BITS, 1), bf),
        "ones1": np.ones((1, BITS), np.float32),
        "wixb": r(S1 * f["ix_w1"][BITS:]),
        "pb": (S1 * (f["ix_w1"][:BITS].T + f["ix_b1"][:, None])
               ).reshape(4, 128, BITS).transpose(1, 0, 2).astype(np.float32),
        "wvb": r(f["v_w1"][BITS:]),
        "vpb": (f["v_w1"][:BITS].T + f["v_b1"][:, None]
                ).reshape(2, 128, BITS).transpose(1, 0, 2).astype(np.float32),
        "wix2": (SW2 * f["ix_w2"]).reshape(4, 128, H).transpose(1, 0, 2).astype(f8),
        "ixb2": (S2 * f["ix_b2"]).reshape(4, 128).T.astype(np.float32),
        "wix3d": (SW3 * np.stack(
            [np.concatenate([f["ix_w3"][k * 128:(k + 1) * 128, :BITS]] * 2, axis=1)
             for k in range(4)], axis=1)).astype(f8),
        # exp bias: ln c = ix_b3, duplicated to both halves
        "ixb3e": np.concatenate([f["ix_b3"][:BITS]] * 2)[:, None].astype(np.float32),
        "dn_w3": np.hstack([np.vstack([
            np.hstack([np.ones((64, 1)), np.zeros((64, 1)), np.zeros((64, 1))]),
            np.hstack([np.zeros((64, 1)), 0.5 * np.ones((64, 1)), np.zeros((64, 1))]),
        ]), np.zeros((128, 29))]).astype(bf),
        "wv2a": np.hstack([np.zeros((128, 2)), f["v_w2"][:128],
                           np.zeros((128, 29))]).astype(bf),
        "wv2b": np.hstack([np.zeros((128, 2)), f["v_w2"][128:],
                           np.zeros((128, 29))]).astype(bf),
        "vb2h": np.full((64, 1), 0.5 * float(f["v_b2"][0]), np.float32),
        "ident": np.eye(64, dtype=np.float32),
    }
    shared = {k: np.ascontiguousarray(v) for k, v in shared.items()}

    in_maps = []
    for c in range(NCORES):
        sb = f["shift_bits"][c * BC:(c + 1) * BC]
        ab = f["a_bits"][c * BC:(c + 1) * BC]
        m = dict(shared)
        m["sbT"] = np.ascontiguousarray(sb.T.astype(bf))
        m["abT"] = np.ascontiguousarray(np.concatenate([ab.T, ab.T], axis=0).astype(bf))
        in_maps.append(m)
    return in_maps


def run(inputs, trace=False):
    nc = _get_program()
    res = bass_utils.run_bass_kernel_spmd(
        nc, make_in_maps(inputs), core_ids=list(range(NCORES)), trace=trace)
    full = np.concatenate([res.results[c]["out"] for c in range(NCORES)], axis=0)
    return full, res


def kernel(**inputs):
    return run(inputs)[0]
